# revision 8
# baseline (speedup 1.0000x reference)
"""Block-sparse local+vertical-stride causal attention for Trainium2 (Bass/Tile).

Problem: B=1, S=2048, H=32, D=128, sparse_block=64, local_blocks=16,
vert_stride=8, head_sliding_step=1. Mask per head h:
  causal(q,k) AND ( (q_blk - k_blk < 16) OR ((k_blk + h + 1) % 8 == 0) )

Sharding: 8 cores; core c computes heads {c, c+8, c+16, c+24}. All four share
the same vertical-stride residue r = (7 - c) % 8, so a single compiled SPMD
program works for every core with per-core *data* (masks + pre-gathered
vertical K/V blocks); the code is identical on all cores.

Device algorithm per (head, q-tile of 256 tokens):
  - scores computed transposed S_T[k, q] = K^T-stationary.T @ Q^T-moving on
    the PE in float32r (full rate at moving dim >= 256)
  - window = up to 10 k-tiles of 128 tokens (16 local blocks + 4 diag-region
    blocks) + 1 pre-gathered vertical tile (blocks {r, r+8})
  - one exp per PSUM chunk on ScalarE (scale = D^-0.5 folded in); no max
    subtraction needed (scores bounded ~20 -> exp well within fp32 range)
  - 0/1 multiplicative masks on VectorE for the q-dependent boundary tiles,
    the token-causal diagonal region and the vertical tile
  - PV: out_T[d, q] += V-stationary.T @ exp_S_T-moving  (no P transpose)
  - denominator via ones-column matmul riding the same PSUM bank as PV,
    reciprocal on DVE, partition-broadcast DMA, normalization folded into the
    PSUM->SBUF copy of out_T
Host reassembles heads and flips the per-head [d, q] layout to [q, d].
"""

import sys
import types

import numpy as np

# ----------------------------------------------------------------------------
# problem constants (hardcoded per contract; kernel.py must be self-contained)
B, S, H, D = 1, 2048, 32, 128
BLOCK = 64
LOCAL = 16
VERT = 8
NCORES = 8
HPC = H // NCORES  # heads per core (4)
QT = 256  # q tokens per window
NT = S // QT  # 8 windows per head
NKT = S // 128  # 16 k-tiles of 128 tokens per head
SCALE = float(D) ** -0.5

# matmul input dtype: "float32r" (full-rate fp32 tensor-engine mode),
# "bfloat16", or "float32" (4x slower, exact)
MM_DT = "float32r"


def _install_ntff_shim():
    """bass_utils wants antenv.axon_hooks (absent in this image); provide it,
    backed by the ctypes NTFF profiler from trn_agent_boot when available."""
    if "antenv.axon_hooks" in sys.modules:
        return
    hook = None
    try:
        from trn_agent_boot.trn_boot import _ntff_profile_via_ctypes

        hook = _ntff_profile_via_ctypes("/opt/axon/libaxon_pjrt.so")
    except Exception:
        hook = None
    m = types.ModuleType("antenv.axon_hooks")
    m.get_axon_ntff_profile_hook = lambda: hook
    m.set_axon_ntff_profile_hook = lambda h: None
    sys.modules["antenv.axon_hooks"] = m


def _i_min(t):
    """First window k-tile index for q-tile t. Tile i covers sparse blocks
    (4t-16+2i, 4t-15+2i), i in [i_min, 10); i in {8, 9} is the diag region."""
    return max(0, 8 - 2 * t)


def _has_vert(t):
    return t >= 5


_NC = None


def _build():
    """Build + compile the per-core Bass program (one NEFF, all cores)."""
    global _NC
    if _NC is not None:
        return _NC
    import concourse.mybir as mybir
    import concourse.tile as tile
    from concourse import bacc

    f32 = mybir.dt.float32
    mdt = getattr(mybir.dt, MM_DT)

    nc = bacc.Bacc("TRN2", target_bir_lowering=False, debug=False,
                   num_devices=NCORES)

    qt_d = nc.dram_tensor("qt", [HPC, D, S], mdt, kind="ExternalInput")
    kt_d = nc.dram_tensor("kt", [HPC, D, S], mdt, kind="ExternalInput")
    v_d = nc.dram_tensor("v", [HPC, S, D], mdt, kind="ExternalInput")
    ktv_d = nc.dram_tensor("ktv", [HPC, D, 128], mdt, kind="ExternalInput")
    vv_d = nc.dram_tensor("vv", [HPC, 128, D], mdt, kind="ExternalInput")
    wm_d = nc.dram_tensor("wmask", [NT, 2, 128, QT], f32, kind="ExternalInput")
    dm_d = nc.dram_tensor("dmask", [2, 128, QT], f32, kind="ExternalInput")
    vm_d = nc.dram_tensor("vmask", [NT, 128], f32, kind="ExternalInput")
    o_d = nc.dram_tensor("o", [HPC, D, S], f32, kind="ExternalOutput")


    with tile.TileContext(nc) as tc:
        with (
            tc.tile_pool(name="consts", bufs=1) as consts,
            tc.tile_pool(name="io", bufs=2) as io,
            tc.tile_pool(name="exps", bufs=2) as exps,
            tc.tile_pool(name="small", bufs=3) as small,
            tc.tile_pool(name="psA", bufs=1, space="PSUM") as psA,
            tc.tile_pool(name="psB", bufs=1, space="PSUM") as psB,
            tc.tile_pool(name="psPV", bufs=2, space="PSUM") as psPV,
        ):
            wmask = consts.tile([128, NT, 2, QT], f32)
            nc.sync.dma_start(out=wmask, in_=wm_d.ap().rearrange("t i p q -> p t i q"))
            dmask = consts.tile([128, 2, QT], f32)
            nc.sync.dma_start(out=dmask, in_=dm_d.ap().rearrange("i p q -> p i q"))
            vmask = consts.tile([128, NT], f32)
            nc.sync.dma_start(out=vmask, in_=vm_d.ap().rearrange("t p -> p t"))
            ones_f32 = consts.tile([128, 1], f32)
            nc.vector.memset(ones_f32, 1.0)
            ones_col = consts.tile([128, 1], mdt)
            nc.vector.tensor_copy(out=ones_col, in_=ones_f32)

            for h in range(HPC):
                qt_sb = io.tile([128, S], mdt, tag="qt")
                nc.sync.dma_start(out=qt_sb, in_=qt_d.ap()[h])
                kt_sb = io.tile([128, S], mdt, tag="kt")
                nc.sync.dma_start(out=kt_sb, in_=kt_d.ap()[h])
                v_sb = io.tile([128, NKT, 128], mdt, tag="v")
                nc.sync.dma_start(
                    out=v_sb, in_=v_d.ap()[h].rearrange("(j p) d -> p j d", p=128)
                )
                ktv_sb = io.tile([128, 128], mdt, tag="ktv")
                nc.sync.dma_start(out=ktv_sb, in_=ktv_d.ap()[h])
                vv_sb = io.tile([128, 128], mdt, tag="vv")
                nc.sync.dma_start(out=vv_sb, in_=vv_d.ap()[h])

                outT = io.tile([128, S], f32, tag="outT")
                rden = io.tile([1, S], f32, tag="rden")

                for t in range(NT):
                    im = _i_min(t)
                    nA = 8 - im  # window tiles i in [im, 8)
                    vert = _has_vert(t)
                    nB = 2 + (1 if vert else 0)
                    q_sl = qt_sb[:, t * QT:(t + 1) * QT]

                    # ---- S_T matmuls -------------------------------------
                    expA = None
                    if nA:
                        sA = psA.tile([128, nA * QT], f32, tag="sA")
                        for a in range(nA):
                            i = im + a
                            toff = 256 * t - 1024 + 128 * i
                            nc.tensor.matmul(
                                sA[:, a * QT:(a + 1) * QT],
                                kt_sb[:, toff:toff + 128],
                                q_sl,
                                start=True, stop=True,
                            )
                    sB = psB.tile([128, nB * QT], f32, tag="sB")
                    for b in range(2):
                        toff = 256 * t + 128 * b
                        nc.tensor.matmul(
                            sB[:, b * QT:(b + 1) * QT],
                            kt_sb[:, toff:toff + 128],
                            q_sl,
                            start=True, stop=True,
                        )
                    if vert:
                        nc.tensor.matmul(
                            sB[:, 2 * QT:3 * QT], ktv_sb, q_sl,
                            start=True, stop=True,
                        )

                    # ---- exp + masks -------------------------------------
                    if nA:
                        expA = exps.tile([128, nA * QT], mdt, tag="expA")
                        nc.scalar.activation(
                            expA, sA, mybir.ActivationFunctionType.Exp,
                            scale=SCALE,
                        )
                        if t >= 4:
                            for i in range(2):
                                nc.vector.tensor_mul(
                                    expA[:, i * QT:(i + 1) * QT],
                                    expA[:, i * QT:(i + 1) * QT],
                                    wmask[:, t, i, :],
                                )
                    expB = exps.tile([128, nB * QT], mdt, tag="expB")
                    nc.scalar.activation(
                        expB, sB, mybir.ActivationFunctionType.Exp, scale=SCALE
                    )
                    for b in range(2):
                        nc.vector.tensor_mul(
                            expB[:, b * QT:(b + 1) * QT],
                            expB[:, b * QT:(b + 1) * QT],
                            dmask[:, b, :],
                        )
                    if vert:
                        nc.vector.tensor_scalar_mul(
                            out=expB[:, 2 * QT:3 * QT],
                            in0=expB[:, 2 * QT:3 * QT],
                            scalar1=vmask[:, t:t + 1],
                        )

                    # ---- PV + denominator matmuls ------------------------
                    # pv tile: cols 0:256 = out_T accumulation, 256:512 = den.
                    # NOTE: the two accumulation chains must NOT interleave —
                    # the PE requires contiguous start/stop groups per region.
                    pv = psPV.tile([128, 512], f32, tag="pv")
                    mm_srcs = []
                    for a in range(nA):
                        i = im + a
                        j = 2 * t - 8 + i  # v k-tile index
                        mm_srcs.append((expA[:, a * QT:(a + 1) * QT],
                                        v_sb[:, j, :]))
                    for b in range(2):
                        mm_srcs.append((expB[:, b * QT:(b + 1) * QT],
                                        v_sb[:, 2 * t + b, :]))
                    if vert:
                        mm_srcs.append((expB[:, 2 * QT:3 * QT], vv_sb))
                    n_mm = len(mm_srcs)
                    for k, (e_sl, v_sl) in enumerate(mm_srcs):
                        nc.tensor.matmul(
                            pv[:, 0:QT], v_sl, e_sl,
                            start=(k == 0), stop=(k == n_mm - 1),
                        )
                    for k, (e_sl, _) in enumerate(mm_srcs):
                        nc.tensor.matmul(
                            pv[0:1, QT:2 * QT], ones_col, e_sl,
                            start=(k == 0), stop=(k == n_mm - 1),
                        )

                    # ---- normalize + stage out ---------------------------
                    nc.vector.reciprocal(
                        out=rden[0:1, t * QT:(t + 1) * QT],
                        in_=pv[0:1, QT:2 * QT],
                    )
                    rbc = small.tile([128, QT], f32, tag="rbc")
                    nc.sync.dma_start(
                        out=rbc,
                        in_=rden[0:1, t * QT:(t + 1) * QT]
                        .unsqueeze(1)
                        .broadcast_to([1, 128, QT]),
                    )
                    nc.vector.tensor_mul(
                        outT[:, t * QT:(t + 1) * QT], pv[:, 0:QT], rbc
                    )

                nc.sync.dma_start(out=o_d.ap()[h], in_=outT)

    nc.compile()
    _NC = nc
    return nc


def _host_prep(query, key, value, core):
    """Per-core input dict. query/key/value: [B, S, H, D] float32 (full)."""
    heads = [core + NCORES * i for i in range(HPC)]
    r = (7 - core) % VERT
    q = query[0][:, heads, :]  # [S, 4, D]
    k = key[0][:, heads, :]
    v = value[0][:, heads, :]
    qt = np.ascontiguousarray(q.transpose(1, 2, 0))  # [4, D, S]
    kt = np.ascontiguousarray(k.transpose(1, 2, 0))
    vn = np.ascontiguousarray(v.transpose(1, 0, 2))  # [4, S, D]
    # vertical gather: k-tokens of blocks {r, r+8}
    vtok = np.concatenate([
        np.arange(r * BLOCK, (r + 1) * BLOCK),
        np.arange((r + 8) * BLOCK, (r + 9) * BLOCK),
    ])
    ktv = np.ascontiguousarray(kt[:, :, vtok])  # [4, D, 128]
    vv = np.ascontiguousarray(vn[:, vtok, :])  # [4, 128, D]

    # masks
    wm = np.ones((NT, 2, 128, QT), dtype=np.float32)
    for t in range(4, NT):
        for i in range(2):
            for ph in range(2):  # partition half -> block
                kb = 4 * t - 16 + 2 * i + ph
                for qb in range(4):
                    act = (qb + 4 * t - kb < LOCAL) or (kb % VERT == r)
                    wm[t, i, ph * 64:(ph + 1) * 64, qb * 64:(qb + 1) * 64] = (
                        1.0 if act else 0.0
                    )
    dm = np.zeros((2, 128, QT), dtype=np.float32)
    for i in range(2):
        for p in range(128):
            k_rel = 128 * i + p
            dm[i, p, k_rel:] = 1.0
    vm = np.zeros((NT, 128), dtype=np.float32)
    for t in range(NT):
        for ph in range(2):
            kb = r + 8 * ph
            if kb < 4 * t - 16:
                vm[t, ph * 64:(ph + 1) * 64] = 1.0

    return {
        "qt": qt, "kt": kt, "v": vn, "ktv": ktv, "vv": vv,
        "wmask": wm, "dmask": dm, "vmask": vm,
    }


def kernel(query, key, value, _trace=False, _tmpdir=None):
    """Full-input entry point: [1, 2048, 32, 128] f32 each -> same shape."""
    _install_ntff_shim()
    from concourse.bass_utils import run_bass_kernel_spmd

    query = np.asarray(query, dtype=np.float32)
    key = np.asarray(key, dtype=np.float32)
    value = np.asarray(value, dtype=np.float32)

    nc = _build()
    in_maps = [_host_prep(query, key, value, c) for c in range(NCORES)]
    res = run_bass_kernel_spmd(
        nc, in_maps, core_ids=list(range(NCORES)),
        trace=_trace, tmpdir=_tmpdir,
    )
    out = np.empty((B, S, H, D), dtype=np.float32)
    for c in range(NCORES):
        o = res.results[c]["o"]  # [4, D, S]
        for i in range(HPC):
            out[0, :, c + NCORES * i, :] = o[i].T
    kernel.last_result = res
    return out
